# revision 8
# baseline (speedup 1.0000x reference)
"""Trainium2 Bass kernel for nn_Attention_38697655337033 (sparse_attention).

GPT-OSS-style sliding-window attention block: QKV proj + YaRN RoPE + GQA
(64 Q heads / 8 KV heads, D=64, window 128, causal) + attention sinks +
o_proj.  Sharded over 8 NeuronCores tensor-parallel by head: core c owns
query heads 8c..8c+7 and KV head c.  o_proj is column-parallel over the
2880 output features (360 per core) after an AllGather of the per-core
attention outputs, so each core emits a disjoint column slice and the host
only concatenates.

Self-contained: hardcodes all shapes; builds and caches the Bass program on
first call.
"""

import math
import sys
import types

import numpy as np

try:
    import concourse.bass as bass  # noqa: F401
except ImportError:  # pragma: no cover
    sys.path.insert(0, "/opt/trn_rl_repo")

import concourse.bass as bass
import concourse.mybir as mybir
import concourse.tile as tile
from concourse.bass_utils import run_bass_kernel_spmd
from concourse.masks import make_identity
from concourse.tile import ScopedClock

# ---------------------------------------------------------------- constants
B, S, E = 1, 1024, 2880
H, KV, D = 64, 8, 64
WIN = 128
BASE, SCALE, ORIG = 150000.0, 32.0, 4096
BFAST, BSLOW = 32.0, 1.0
SCALING = D ** -0.5  # 0.125, exact power of two -> folded into Wq on host

N_CORES = 8
HL = H // N_CORES          # 8 local query heads
HD_L = HL * D              # 512 local q dims
EC = E // N_CORES          # 360 output columns per core
EP = 2944                  # E padded to 23*128
KT = EP // 128             # 23 contraction tiles for projections
NQT = HL // 2              # 4 head-pair tiles
NQW = S // 256             # 4 query windows of 256
NSB = S // 128             # 8 seq blocks of 128
KO = H * D // 128          # 32 o_proj contraction tiles

FP32 = mybir.dt.float32
FP32R = mybir.dt.float32r
MM_DT = FP32R              # matmul operand dtype (bitcast)

# ------------------------------------------------------- walrus compat patch
# This container's walrus build rejects instructions with >1 sync-wait
# ("Too many sync wait commands").  Split extra waits onto same-engine NoOp
# carriers, and split the final Tile drain into one drain per wait.
_compat_done = [False]
_carrier_n = [0]


def _install_tile_compat():
    if _compat_done[0]:
        return
    _compat_done[0] = True

    orig_cal = tile.TileContext._commit_and_lower

    def patched_cal(self, inst, original_block, old_bb_map, bb_to_exit_bb):
        if isinstance(inst, mybir.Instruction):
            si = getattr(inst, "sync_info", None)
            if si is not None and len(si.on_wait) > 1:
                waits = list(si.on_wait)
                for w in waits[:-1]:
                    _carrier_n[0] += 1
                    nop = mybir.InstNoOp(
                        name=f"swsplit-{_carrier_n[0]}",
                        engine=inst.engine,
                        sync_info=mybir.SyncInfo(on_wait=[w], on_update=[]),
                        bass_nofuse=True,
                    )
                    self._commit_instruction(nop)
                inst.sync_info = mybir.SyncInfo(
                    on_wait=[waits[-1]], on_update=list(si.on_update)
                )
        return orig_cal(self, inst, original_block, old_bb_map, bb_to_exit_bb)

    tile.TileContext._commit_and_lower = patched_cal

    def patched_dab(self, tick_clock, wait_clock):
        drain_inst = self.nc.sync.drain()
        wait_clock.add_sem_waits(
            drain_inst.ins, ScopedClock({None: tick_clock.global_clock})
        )
        si = drain_inst.ins.sync_info
        if si is not None and len(si.on_wait) > 1:
            waits = list(si.on_wait)
            drain_inst.ins.sync_info = mybir.SyncInfo(on_wait=waits[:1], on_update=[])
            for i in range(1, len(waits)):
                extra = self.nc.sync.drain()
                extra.ins.sync_info = mybir.SyncInfo(
                    on_wait=waits[i : i + 1], on_update=[]
                )
        self.nc.all_engine_barrier()
        assert self.sems is not None
        popped = self.nc._tile_sem_poison_stack.pop()
        assert popped is self._sem_poison
        self.nc.clear_and_free_semaphores(list(self.sems.allocated().values()))
        self.nc.all_engine_barrier()

    tile.TileContext._drain_and_barrier = patched_dab


def _install_prof_shim():
    """antenv.axon_hooks is missing in this container; provide it so
    BASS_TRACE-style profiling paths don't crash."""
    try:
        import antenv.axon_hooks  # noqa: F401
        return
    except ImportError:
        pass
    try:
        import antenv
        from trn_agent_boot.trn_boot import _ntff_profile_via_ctypes

        hook = _ntff_profile_via_ctypes("/opt/axon/libaxon_pjrt.so")
    except Exception:
        hook = None
        try:
            import antenv
        except ImportError:
            return
    mod = types.ModuleType("antenv.axon_hooks")
    mod._hook = hook
    mod.get_axon_ntff_profile_hook = lambda: mod._hook

    def _set(h):
        mod._hook = h

    mod.set_axon_ntff_profile_hook = _set
    sys.modules["antenv.axon_hooks"] = mod
    antenv.axon_hooks = mod


# ---------------------------------------------------------------- host prep
def _rope_tables_np(positions):
    """cos/sin YaRN tables, mirroring the reference, in float32."""
    def find_dim(rot):
        return D * math.log(ORIG / (rot * 2 * math.pi)) / (2 * math.log(BASE))

    low = max(find_dim(BFAST), 0.0)
    high = min(find_dim(BSLOW), D // 2 - 1)
    if low == high:
        high += 0.001
    pos_freqs = (BASE ** (np.arange(0, D, 2, dtype=np.float32) / np.float32(D))).astype(
        np.float32
    )
    ramp = np.clip(
        (np.arange(D // 2, dtype=np.float32) - np.float32(low))
        / np.float32(high - low),
        0.0,
        1.0,
    ).astype(np.float32)
    inv_freq = (
        (np.float32(1.0) / (np.float32(SCALE) * pos_freqs)) * ramp
        + (np.float32(1.0) / pos_freqs) * (np.float32(1.0) - ramp)
    ).astype(np.float32)
    mscale = np.float32(0.1 * math.log(SCALE) + 1.0)
    ang = positions.astype(np.float32)[:, None] * inv_freq[None, :]  # [S, 32]
    emb = np.concatenate([ang, ang], axis=-1)  # [S, D]
    return (np.cos(emb) * mscale).astype(np.float32), (np.sin(emb) * mscale).astype(
        np.float32
    )


def _make_masks():
    """Multiplicative [128, 768] masks in the transposed-score layout.

    Slot s (of 3) covers key block kb = 2Q-1+s for query window Q (256 wide).
    Entry [j, i2] is 1 when query i2 may attend key j of that block:
      slot0: i2 <  j           (keys one block behind the window)
      slot1: j <= i2 <= j+127  (keys in the window's first block)
      slot2: i2 >= j+128       (keys in the window's second block)
    For Q=0 slot0's block doesn't exist -> zeros (mask_q0).
    """
    j = np.arange(128)[:, None]
    i2 = np.arange(256)[None, :]
    m0 = (i2 < j).astype(np.float32)
    m1 = ((i2 >= j) & (i2 <= j + 127)).astype(np.float32)
    m2 = (i2 >= j + 128).astype(np.float32)
    maskn = np.concatenate([m0, m1, m2], axis=1)
    maskq0 = np.concatenate([np.zeros_like(m0), m1, m2], axis=1)
    return maskn, maskq0


def host_prepare(hidden_states, positions, Wq, bq, Wk, bk, Wv, bv, Wo, bo, sinks):
    """Build the 8 per-core input maps (all float32 numpy)."""
    x = np.asarray(hidden_states, np.float32).reshape(S, E)
    xT = np.zeros((EP, S), np.float32)
    xT[:E] = np.ascontiguousarray(x.T)

    cos, sin = _rope_tables_np(np.asarray(positions))
    cosT = np.ascontiguousarray(cos.T)  # [64, S]
    sinT = np.ascontiguousarray(sin.T)
    sgn = np.where(np.arange(D) < D // 2, np.float32(-1.0), np.float32(1.0))
    sinTs = sinT * sgn[:, None]
    cos2 = np.concatenate([cosT, cosT], axis=0)  # [128, S]
    sin2s = np.concatenate([sinTs, sinTs], axis=0)

    maskn, maskq0 = _make_masks()

    Wq = np.asarray(Wq, np.float32)
    Wk = np.asarray(Wk, np.float32)
    Wv = np.asarray(Wv, np.float32)
    Wo = np.asarray(Wo, np.float32)
    bq = np.asarray(bq, np.float32)
    bk = np.asarray(bk, np.float32)
    bv = np.asarray(bv, np.float32)
    bo = np.asarray(bo, np.float32)
    sinks = np.asarray(sinks, np.float32)

    in_maps = []
    for c in range(N_CORES):
        wq_c = Wq[c * HD_L : (c + 1) * HD_L] * np.float32(SCALING)  # [512, E]
        wq_dev = np.zeros((EP, HD_L), np.float32)
        wq_dev[:E] = wq_c.T
        wkv_c = np.concatenate(
            [Wk[c * D : (c + 1) * D], Wv[c * D : (c + 1) * D]], axis=0
        )  # [128, E]
        wkv_dev = np.zeros((EP, 128), np.float32)
        wkv_dev[:E] = wkv_c.T

        bq_c = (bq[c * HD_L : (c + 1) * HD_L] * np.float32(SCALING)).reshape(4, 128)
        bq_dev = np.ascontiguousarray(bq_c.T)  # [128, 4]
        bkv_dev = np.concatenate(
            [bk[c * D : (c + 1) * D], bv[c * D : (c + 1) * D]]
        ).reshape(128, 1)

        wo_dev = np.ascontiguousarray(Wo[c * EC : (c + 1) * EC, :].T)  # [4096, 360]
        bo_dev = np.ascontiguousarray(np.broadcast_to(bo[c * EC : (c + 1) * EC].reshape(1, EC), (128, EC)))
        esink = np.exp(sinks[c * HL : (c + 1) * HL]).reshape(1, HL).astype(np.float32)

        sel64_np = np.zeros((65, 128), np.float32)
        sel64_np[64, :] = 1.0
        in_maps.append(
            {
                "sel64": sel64_np,
                "ones8": np.ones((128, NSB), np.float32),
                "xT": xT,
                "wq": np.ascontiguousarray(wq_dev),
                "wkv": np.ascontiguousarray(wkv_dev),
                "wo": wo_dev,
                "bq": bq_dev,
                "bkv": np.ascontiguousarray(bkv_dev),
                "bo": np.ascontiguousarray(bo_dev),
                "cos2": np.ascontiguousarray(cos2),
                "sin2s": np.ascontiguousarray(sin2s),
                "maskn": np.ascontiguousarray(maskn),
                "maskq0": np.ascontiguousarray(maskq0),
                "esink": esink,
            }
        )
    return in_maps


# ------------------------------------------------------------- device build
def build_program():
    _install_tile_compat()
    _install_prof_shim()

    nc = bass.Bass("TRN2", target_bir_lowering=False, debug=False, num_devices=N_CORES)

    xT = nc.declare_dram_parameter("xT", [EP, S], MM_DT, isOutput=False)
    wq = nc.declare_dram_parameter("wq", [EP, HD_L], MM_DT, isOutput=False)
    wkv = nc.declare_dram_parameter("wkv", [EP, 128], MM_DT, isOutput=False)
    wo = nc.declare_dram_parameter("wo", [H * D, EC], MM_DT, isOutput=False)
    bq = nc.declare_dram_parameter("bq", [128, 4], FP32, isOutput=False)
    bkv = nc.declare_dram_parameter("bkv", [128, 1], FP32, isOutput=False)
    bo = nc.declare_dram_parameter("bo", [128, EC], FP32, isOutput=False)
    cos2 = nc.declare_dram_parameter("cos2", [128, S], FP32, isOutput=False)
    sin2s = nc.declare_dram_parameter("sin2s", [128, S], FP32, isOutput=False)
    maskn_d = nc.declare_dram_parameter("maskn", [128, 768], FP32, isOutput=False)
    maskq0_d = nc.declare_dram_parameter("maskq0", [128, 768], FP32, isOutput=False)
    esink_d = nc.declare_dram_parameter("esink", [1, HL], FP32, isOutput=False)
    sel64_d = nc.declare_dram_parameter("sel64", [65, 128], MM_DT, isOutput=False)
    ones_d = nc.declare_dram_parameter("ones8", [128, NSB], MM_DT, isOutput=False)
    y = nc.declare_dram_parameter("y", [S, EC], FP32, isOutput=True)

    Ident = mybir.ActivationFunctionType.Identity
    Exp = mybir.ActivationFunctionType.Exp
    Mult = mybir.AluOpType.mult
    Add = mybir.AluOpType.add

    with tile.TileContext(nc) as tc, nc.allow_low_precision(
        reason="float32r operands for PE fast path; accumulation stays fp32"
    ):

        with tc.tile_pool(name="persist", bufs=1) as per:
            # persistent SBUF state
            qT = per.tile([128, NQT, S], MM_DT)        # rope'd qT, head-pair tiles
            k2T = per.tile([128, S], MM_DT)            # kT duplicated on both halves
            v_sb = per.tile([128, NSB, 66], MM_DT)     # v natural + ones column
            attnT = per.tile([128, NQT, S], MM_DT)     # normalized attention out.T
            cos_sb = per.tile([128, S], FP32)
            sin_sb = per.tile([128, S], FP32)
            maskn_sb = per.tile([128, 768], FP32)
            maskq0_sb = per.tile([128, 768], FP32)
            bq_sb = per.tile([128, 4], FP32)
            bkv_sb = per.tile([128, 1], FP32)
            esink_sb = per.tile([1, HL], FP32)
            bo_sb = per.tile([128, EC], FP32)
            ident = per.tile([128, 128], FP32)
            sel64 = per.tile([65, 128], MM_DT)

            nc.sync.dma_start(cos_sb[:], cos2[:])
            nc.sync.dma_start(sin_sb[:], sin2s[:])
            nc.sync.dma_start(maskn_sb[:], maskn_d[:])
            nc.sync.dma_start(maskq0_sb[:], maskq0_d[:])
            nc.sync.dma_start(bq_sb[:], bq[:])
            nc.sync.dma_start(bkv_sb[:], bkv[:])
            nc.sync.dma_start(esink_sb[:], esink_d[:])
            nc.sync.dma_start(bo_sb[:], bo[:])
            make_identity(nc, ident[:])
            nc.sync.dma_start(sel64[:], sel64_d[:])

            # ---------------------------------------------- phase 1: QKV proj
            with (
                tc.tile_pool(name="wpool", bufs=1) as wpool,
                tc.tile_pool(name="xpool", bufs=4) as xpool,
                tc.tile_pool(name="qb_pool", bufs=1) as qbp,
                tc.tile_pool(name="shufp", bufs=2) as shufp,
                tc.tile_pool(name="pproj", bufs=1, space="PSUM") as pproj,
            ):
                wq_t = [wpool.tile([128, HD_L], MM_DT, name=f"wqt{k}") for k in range(KT)]
                wkv_t = [wpool.tile([128, 128], MM_DT, name=f"wkvt{k}") for k in range(KT)]
                for k in range(KT):
                    nc.sync.dma_start(wq_t[k][:], wq[k * 128 : (k + 1) * 128, :])
                    nc.sync.dma_start(wkv_t[k][:], wkv[k * 128 : (k + 1) * 128, :])

                qb = qbp.tile([128, NQT, S], FP32)
                kvb = qbp.tile([128, S], FP32)

                for half in range(2):
                    sl = slice(half * 512, (half + 1) * 512)
                    ps_q = [
                        pproj.tile([128, 512], FP32, name=f"psq{t}", tag=f"psq{t}")
                        for t in range(NQT)
                    ]
                    ps_kv = pproj.tile([128, 512], FP32, name="pskv", tag="pskv")
                    for k in range(KT):
                        xk = xpool.tile([128, 512], MM_DT, name="xk", tag="xk")
                        nc.sync.dma_start(xk[:], xT[k * 128 : (k + 1) * 128, sl])
                        xkr = xk[:]
                        st = k == 0
                        sp = k == KT - 1
                        for t in range(NQT):
                            nc.tensor.matmul(
                                ps_q[t][:],
                                wq_t[k][:, t * 128 : (t + 1) * 128],
                                xkr,
                                start=st,
                                stop=sp,
                            )
                        nc.tensor.matmul(
                            ps_kv[:], wkv_t[k][:], xkr, start=st, stop=sp
                        )
                    # evacuate with bias add
                    for t in range(NQT):
                        nc.scalar.activation(
                            qb[:, t, sl], ps_q[t][:], Ident, bias=bq_sb[:, t : t + 1]
                        )
                    nc.scalar.activation(kvb[:, sl], ps_kv[:], Ident, bias=bkv_sb[:, 0:1])

                # ------------------------------------------------ RoPE on q
                for t in range(NQT):
                    qs = shufp.tile([128, S], FP32, name="qs", tag="qs")
                    nc.sync.dma_start(qs[0:32, :], qb[32:64, t, :])
                    nc.sync.dma_start(qs[32:64, :], qb[0:32, t, :])
                    nc.sync.dma_start(qs[64:96, :], qb[96:128, t, :])
                    nc.sync.dma_start(qs[96:128, :], qb[64:96, t, :])
                    nc.vector.tensor_tensor(qs[:], qs[:], sin_sb[:], Mult)
                    nc.vector.tensor_tensor(qT[:, t, :], qb[:, t, :], cos_sb[:], Mult)
                    nc.vector.tensor_tensor(qT[:, t, :], qT[:, t, :], qs[:], Add)

                # ------------------------------------------------ RoPE on k
                ks = shufp.tile([128, S], FP32, name="ks", tag="qs")
                nc.sync.dma_start(ks[0:32, :], kvb[32:64, :])
                nc.sync.dma_start(ks[32:64, :], kvb[0:32, :])
                nc.vector.tensor_tensor(ks[0:64, :], ks[0:64, :], sin_sb[0:64, :], Mult)
                nc.vector.tensor_tensor(
                    k2T[0:64, :], kvb[0:64, :], cos_sb[0:64, :], Mult
                )
                nc.vector.tensor_tensor(k2T[0:64, :], k2T[0:64, :], ks[0:64, :], Add)
                nc.sync.dma_start(k2T[64:128, :], k2T[0:64, :])

                # ------------------------------------- v transpose to natural
                ps_vt = pproj.tile([128, 64], FP32, name="psvt", tag="psvt")
                for sbk in range(NSB):
                    ps_vt_i = pproj.tile([128, 64], FP32, name="psvt", tag="psvt")
                    nc.tensor.transpose(
                        ps_vt_i[:],
                        kvb[64:128, sbk * 128 : (sbk + 1) * 128],
                        ident[64:128, 64:128],
                    )
                    nc.vector.tensor_copy(v_sb[:, sbk, 0:64], ps_vt_i[:])
                nc.sync.dma_start(v_sb[:, :, 64:65], ones_d[:, :, None])

            # ---------------------------------------------- phase 2: attention
            with (
                tc.tile_pool(name="ppool", bufs=3) as ppool,
                tc.tile_pool(name="rnpool", bufs=4) as rnpool,
                tc.tile_pool(name="ps_s", bufs=2, space="PSUM") as ps_s_pool,
                tc.tile_pool(name="ps_o", bufs=2, space="PSUM") as ps_o_pool,
            ):
                for t in range(NQT):
                    for qw in range(NQW):
                        qsl = slice(qw * 256, (qw + 1) * 256)
                        ps_sA = ps_s_pool.tile([128, 768], FP32, name="pssA", tag="pss")
                        ps_sB = ps_s_pool.tile([128, 768], FP32, name="pssB", tag="pss")
                        for slot in range(3):
                            kb = min(max(2 * qw - 1 + slot, 0), 7)
                            ksl = slice(kb * 128, (kb + 1) * 128)
                            osl = slice(slot * 256, (slot + 1) * 256)
                            nc.tensor.matmul(
                                ps_sA[:, osl],
                                k2T[0:64, ksl],
                                qT[0:64, t, qsl],
                                start=True,
                                stop=True,
                            )
                            nc.tensor.matmul(
                                ps_sB[:, osl],
                                k2T[64:128, ksl],
                                qT[64:128, t, qsl],
                                start=True,
                                stop=True,
                            )
                        mask = maskq0_sb if qw == 0 else maskn_sb
                        pA = ppool.tile([128, 768], MM_DT, name="pA", tag="pp")
                        pB = ppool.tile([128, 768], MM_DT, name="pB", tag="pp")
                        nc.scalar.activation(pA[:], ps_sA[:], Exp)
                        nc.scalar.activation(pB[:], ps_sB[:], Exp)
                        nc.vector.tensor_tensor(pA[:], pA[:], mask[:], Mult)
                        nc.vector.tensor_tensor(pB[:], pB[:], mask[:], Mult)

                        ps_oA = ps_o_pool.tile([65, 256], FP32, name="psoA", tag="pso")
                        ps_oB = ps_o_pool.tile([65, 256], FP32, name="psoB", tag="pso")
                        for slot in range(3):
                            kb = min(max(2 * qw - 1 + slot, 0), 7)
                            osl = slice(slot * 256, (slot + 1) * 256)
                            nc.tensor.matmul(
                                ps_oA[:],
                                v_sb[:, kb, 0:65],
                                pA[:, osl],
                                start=slot == 0,
                                stop=slot == 2,
                            )
                            nc.tensor.matmul(
                                ps_oB[:],
                                v_sb[:, kb, 0:65],
                                pB[:, osl],
                                start=slot == 0,
                                stop=slot == 2,
                            )
                        dsbAB = rnpool.tile([65, 512], MM_DT, name="dsbAB", tag="dsbAB")
                        rbAB = rnpool.tile([128, 512], FP32, name="rbAB", tag="rbAB")
                        ps_rb = ps_o_pool.tile(
                            [128, 512], FP32, name="ps_rb", tag="ps_rb"
                        )
                        nc.vector.tensor_scalar(
                            dsbAB[64:65, 0:256],
                            ps_oA[64:65, :],
                            esink_sb[0:1, 2 * t : 2 * t + 1],
                            None,
                            Add,
                        )
                        nc.vector.tensor_scalar(
                            dsbAB[64:65, 256:512],
                            ps_oB[64:65, :],
                            esink_sb[0:1, 2 * t + 1 : 2 * t + 2],
                            None,
                            Add,
                        )
                        nc.vector.reciprocal(dsbAB[64:65, :], dsbAB[64:65, :])
                        nc.tensor.matmul(
                            ps_rb[:], sel64[:], dsbAB[:], start=True, stop=True
                        )
                        nc.scalar.activation(rbAB[:], ps_rb[:], Ident)
                        nc.vector.tensor_tensor(
                            attnT[0:64, t, qsl],
                            ps_oA[0:64, :],
                            rbAB[0:64, 0:256],
                            Mult,
                        )
                        nc.vector.tensor_tensor(
                            attnT[64:128, t, qsl],
                            ps_oB[0:64, :],
                            rbAB[64:128, 256:512],
                            Mult,
                        )

            # ------------------------------------------ phase 3: AllGather
            with tc.tile_pool(name="dram", bufs=1, space="DRAM") as dram:
                ag_in = dram.tile([HD_L, S], MM_DT)
                ag_out = dram.tile([H * D, S], MM_DT, addr_space="Shared")
                nc.sync.dma_start(
                    ag_in[:].rearrange("(t p) s -> p t s", p=128), attnT[:]
                )
                nc.gpsimd.collective_compute(
                    "AllGather",
                    mybir.AluOpType.bypass,
                    ins=[ag_in[:].opt()],
                    outs=[ag_out[:].opt()],
                    replica_groups=[list(range(N_CORES))],
                )

                # -------------------------------------- phase 4: o_proj
                with (
                    tc.tile_pool(name="at_pool", bufs=3) as atp,
                    tc.tile_pool(name="wo_pool", bufs=3) as wop,
                    tc.tile_pool(name="out_pool", bufs=3) as outp,
                    tc.tile_pool(name="ps_out", bufs=1, space="PSUM") as ps_out_pool,
                ):
                    ps_out = [
                        ps_out_pool.tile([128, EC], FP32, name=f"pso{s}", tag=f"pso{s}")
                        for s in range(NSB)
                    ]
                    for k in range(KO):
                        at_k = atp.tile([128, S], MM_DT, name="at_k", tag="at")
                        wo_k = wop.tile([128, EC], MM_DT, name="wo_k", tag="wo")
                        nc.sync.dma_start(at_k[:], ag_out[k * 128 : (k + 1) * 128, :])
                        nc.sync.dma_start(wo_k[:], wo[k * 128 : (k + 1) * 128, :])
                        wr = wo_k[:]
                        for s in range(NSB):
                            nc.tensor.matmul(
                                ps_out[s][:],
                                at_k[:, s * 128 : (s + 1) * 128],
                                wr,
                                start=k == 0,
                                stop=k == KO - 1,
                            )
                    for s in range(NSB):
                        out_s = outp.tile([128, EC], FP32, name="out_s", tag="out")
                        nc.vector.tensor_tensor(out_s[:], ps_out[s][:], bo_sb[:], Add)
                        nc.sync.dma_start(y[s * 128 : (s + 1) * 128, :], out_s[:])

    return nc


_PROGRAM = [None]


def _get_program():
    if _PROGRAM[0] is None:
        _PROGRAM[0] = build_program()
    return _PROGRAM[0]


def kernel(**inputs) -> np.ndarray:
    nc = _get_program()
    in_maps = host_prepare(**inputs)
    res = run_bass_kernel_spmd(nc, in_maps, list(range(N_CORES)))
    out = np.concatenate([res.results[c]["y"] for c in range(N_CORES)], axis=1)
    return out.reshape(B, S, E)


def kernel_traced(tmpdir=None, **inputs):
    """Like kernel() but with NTFF profiling; returns (out, BassKernelResults)."""
    _install_prof_shim()
    from concourse import bass_utils

    bass_utils.upload_artifacts = lambda d: str(d)
    nc = _get_program()
    in_maps = host_prepare(**inputs)
    res = run_bass_kernel_spmd(
        nc, in_maps, list(range(N_CORES)), trace=True, tmpdir=tmpdir
    )
    out = np.concatenate([res.results[c]["y"] for c in range(N_CORES)], axis=1)
    return out.reshape(B, S, E), res
